# revision 15
# baseline (speedup 1.0000x reference)
"""2-layer GCN encoder on 8 Trainium2 NeuronCores (Bass/Tile).

Strategy: nodes sharded across 8 cores; edges partitioned by destination
node; per-layer feature tables AllGathered in two loc-stripes; scatter-add
done locally per dst shard via one-hot matmuls on the tensor engine.

Math: GCNConv's symmetric norm factorizes: norm(e) = dinv[src]*dinv[dst].
With g = dinv * (x @ W), the aggregation is out[d] = dinv[d]*(g[d] +
sum_{e:dst=d} g[src(e)]) + b (the g[d] term is the self loop, applied from
local SBUF data via an identity matmul -- never gathered).

Perf structure:
  - gather calls are 1024 rows each (the ucode max), 4 SWDGE queues
  - self loops use local data (6% fewer gather descriptors)
  - the node table is split into two loc-stripes, each AllGathered
    separately; layer-2 AllGathers are chunked (3 pieces) so they overlap
    the tail of the layer-1 consume loop
  - gather indices are loaded once and reused by both layers
  - layer-2 self rows (g2) stay resident in SBUF from the layer-1 epilogue
"""

import sys

for _p in ("/opt/trn_rl_repo", "/opt/trn_rl_repo/concourse"):
    if _p not in sys.path:
        sys.path.insert(0, _p)

import numpy as np

import concourse.bacc as bacc
import concourse.mybir as mybir
import concourse.tile as tile
from concourse.bass_utils import run_bass_kernel_spmd
from concourse.library_config import mlp as _mlp_lib

F32 = mybir.dt.float32
F32R = mybir.dt.float32r
BF16 = mybir.dt.bfloat16
I16 = mybir.dt.int16
AF = mybir.ActivationFunctionType
ALU = mybir.AluOpType

C = 8          # cores
P = 128        # partitions / window size
NSW = 2        # table stripes
CALLCH = 8     # chunks (of 128 rows) per dma_gather call (ucode max 1024)
LOOKAHEAD = 6    # windows of gather issue-ahead
PAYBUFS = (10, 8)   # layer-1 payload ring depth per stripe
PAYBUFS2 = (12, 8)  # layer-2 payload ring depth per stripe


def _ceil(a, b):
    return (a + b - 1) // b


class Plan:
    """Static structure shared by all cores (program shape)."""

    def __init__(self, N, E, DIN, DH, DOUT):
        self.N, self.E = N, E
        self.DIN, self.DH, self.DOUT = DIN, DH, DOUT
        self.NLOC = N // C
        self.NT = _ceil(self.NLOC, P)
        self.NLOC_PAD = self.NT * P
        assert self.NLOC_PAD % NSW == 0
        self.SW = self.NLOC_PAD // NSW          # stripe width (locs)
        self.STBL = C * self.SW                 # rows per stripe table
        assert self.STBL <= 32768
        self.CH = None        # [NT, NSW] common chunk counts
        self.offs = None      # [NSW][NT] chunk offsets within stream
        self.woff = None      # [NT] dstrel column offset of window w
        self.LS = None        # [NSW] stream rows
        self.TOTCH = None
        self.CHMAX = None
        self.calls = None     # [NSW] list of (chunk0, nch, first_w)

    def key(self):
        return (self.N, self.E, self.CH.tobytes())


def preprocess(x, edge_index, W1, b1, W2, b2):
    """Host-side sharding: integer index metadata only (no float compute on
    the feature data beyond dtype casts / layout of the given tensors)."""
    x = np.asarray(x)
    N, DIN = x.shape
    E = edge_index.shape[1]
    DH = W1.shape[1]
    DOUT = W2.shape[1]
    plan = Plan(N, E, DIN, DH, DOUT)
    NLOC, NT, SW = plan.NLOC, plan.NT, plan.SW

    src = np.asarray(edge_index[0], dtype=np.int64)
    dst = np.asarray(edge_index[1], dtype=np.int64)

    # degree with self loop -- integer graph metadata
    deg = (np.bincount(dst, minlength=N) + 1).astype(np.float32)

    co = dst // NLOC                     # owning core (by dst)
    dl = dst - co * NLOC
    w_of = dl // P
    rel = (dl % P).astype(np.int16)
    sc = src // NLOC
    sl = src - sc * NLOC
    stripe = sl // SW                     # 0/1
    rowidx = (sc * SW + sl - stripe * SW).astype(np.int16)

    # group edges by (core, window, stripe)
    gid = (co * NT + w_of) * NSW + stripe
    order = np.argsort(gid, kind="stable")
    gid_s = gid[order]
    row_s = rowidx[order]
    rel_s = rel[order]
    n_groups = C * NT * NSW
    counts = np.bincount(gid_s, minlength=n_groups).reshape(C, NT, NSW)
    starts = np.concatenate([[0], np.cumsum(counts.reshape(-1))[:-1]]).reshape(
        C, NT, NSW)

    # common chunk structure = max over cores
    CH = np.ceil(counts.max(axis=0) / P).astype(np.int64)      # [NT, NSW]
    plan.CH = CH
    plan.offs = [np.concatenate([[0], np.cumsum(CH[:, s])[:-1]]) for s in range(NSW)]
    plan.LS = [int(CH[:, s].sum()) * P for s in range(NSW)]
    chw = CH.sum(axis=1)
    plan.woff = np.concatenate([[0], np.cumsum(chw)[:-1]])
    plan.TOTCH = int(chw.sum())
    plan.CHMAX = int(chw.max())

    # call packing per stripe (common across cores)
    plan.calls = []
    for s in range(NSW):
        nch = int(CH[:, s].sum())
        calls = []
        c0 = 0
        wof = np.repeat(np.arange(NT), CH[:, s])
        while c0 < nch:
            n = min(CALLCH, nch - c0)
            calls.append((c0, n, int(wof[c0])))
            c0 += n
        plan.calls.append(calls)

    def wrap_idx(a):
        # [n] -> [16, n//16] column-major wrap, replicated x8 -> [128, n//16]
        n = a.shape[0]
        w = a.reshape(n // 16, 16).T
        return np.ascontiguousarray(np.tile(w, (8, 1)))

    in_maps = []
    for c in range(C):
        lo = c * NLOC
        # x, pre-transposed to [128, NT*DIN]: col w*DIN+f, row p = node w*P+p
        xpad = np.zeros((plan.NLOC_PAD, DIN), np.float32)
        xpad[:NLOC] = x[lo:lo + NLOC]
        xshT = np.ascontiguousarray(
            xpad.reshape(NT, P, DIN).transpose(1, 0, 2).reshape(P, NT * DIN))

        degf = np.ones((plan.NLOC_PAD,), np.float32)
        degf[:NLOC] = deg[lo:lo + NLOC]
        degf = np.ascontiguousarray(degf.reshape(NT, P).T)   # [P, NT]

        idxs = [np.zeros((plan.LS[s],), np.int16) for s in range(NSW)]
        drel = np.full((plan.TOTCH * P,), -1, np.int16)
        for w in range(NT):
            for s in range(NSW):
                n = counts[c, w, s]
                st = starts[c, w, s]
                p0 = plan.offs[s][w] * P
                idxs[s][p0:p0 + n] = row_s[st:st + n]
                col0 = plan.woff[w] + (CH[w, 0] if s == 1 else 0)
                drel[col0 * P:col0 * P + n] = rel_s[st:st + n]

        in_maps.append({
            "xshT": xshT,
            "W1": np.asarray(W1, np.float32),
            "W2": np.asarray(W2, np.float32),
            "b1r": np.tile(np.asarray(b1, np.float32)[None, :], (P, 1)),
            "b2r": np.tile(np.asarray(b2, np.float32)[None, :], (P, 1)),
            "degf": degf,
            "idx0": wrap_idx(idxs[0]),
            "idx1": wrap_idx(idxs[1]),
            "dstrel": np.ascontiguousarray(drel.reshape(plan.TOTCH, P).T),
        })
    return plan, in_maps


def build(plan: Plan):
    DIN, DH, DOUT = plan.DIN, plan.DH, plan.DOUT
    NT, SW, STBL = plan.NT, plan.SW, plan.STBL
    NLOC = plan.NLOC
    CH, offs, woff = plan.CH, plan.offs, plan.woff
    CHMAX = plan.CHMAX

    nc = bacc.Bacc("TRN2", target_bir_lowering=False, debug=False, num_devices=C,
                   dynamic_dma_scratch_size=32768, num_swdge_queues=4)

    xshT = nc.dram_tensor("xshT", [P, NT * DIN], F32, kind="ExternalInput")
    W1 = nc.dram_tensor("W1", [DIN, DH], F32R, kind="ExternalInput")
    W2 = nc.dram_tensor("W2", [DH, DOUT], F32R, kind="ExternalInput")
    b1r = nc.dram_tensor("b1r", [P, DH], F32, kind="ExternalInput")
    b2r = nc.dram_tensor("b2r", [P, DOUT], F32, kind="ExternalInput")
    degf = nc.dram_tensor("degf", [P, NT], F32, kind="ExternalInput")
    idx_d = [nc.dram_tensor(f"idx{s}", [P, plan.LS[s] // 16], I16,
                            kind="ExternalInput") for s in range(NSW)]
    dstrel = nc.dram_tensor("dstrel", [P, plan.TOTCH], I16, kind="ExternalInput")
    out = nc.dram_tensor("out", [NLOC, DOUT], F32, kind="ExternalOutput")

    g1_in = [nc.dram_tensor(f"g1_in{s}", [SW, DIN], BF16) for s in range(NSW)]
    g1_tbl = [nc.dram_tensor(f"g1_tbl{s}", [STBL, DIN], BF16, addr_space="Shared")
              for s in range(NSW)]
    g2_in = [nc.dram_tensor(f"g2_in{s}", [SW, DOUT], BF16) for s in range(NSW)]
    g2_tbl = [nc.dram_tensor(f"g2_tbl{s}", [STBL, DOUT], BF16, addr_space="Shared")
              for s in range(NSW)]

    with tile.TileContext(nc) as tc:
        with tc.tile_pool(name="const", bufs=1) as cpool, \
             tc.tile_pool(name="sbuild", bufs=3) as spool, \
             tc.tile_pool(name="epi", bufs=3) as epool:

            nc.gpsimd.load_library(_mlp_lib)

            # ---- degree -> dinv, then x -> g1 table production first (so the
            # stripe AllGathers start as early as possible) ----
            deg_sb = cpool.tile([P, NT], F32, tag="deg")
            nc.sync.dma_start(deg_sb[:, :], degf[:, :])
            sq_sb = cpool.tile([P, NT], F32, tag="sqdeg")
            nc.scalar.activation(sq_sb[:, :], deg_sb[:, :], AF.Sqrt)
            dinv_sb = cpool.tile([P, NT], F32, tag="dinv")
            nc.vector.reciprocal(dinv_sb[:, :], sq_sb[:, :])

            def store_win(g_dst, g_tile, w):
                """Store window w ([P, dim] sbuf rows) into per-stripe dram."""
                r0 = w * P
                s0 = r0 // SW
                s1 = (r0 + P - 1) // SW
                if s0 == s1:
                    nc.sync.dma_start(g_dst[s0][r0 - s0 * SW:r0 - s0 * SW + P, :],
                                      g_tile)
                else:
                    cut = s1 * SW - r0
                    nc.sync.dma_start(g_dst[s0][r0 - s0 * SW:, :], g_tile[:cut, :])
                    nc.sync.dma_start(g_dst[s1][0:P - cut, :], g_tile[cut:, :])

            # g1 rows stay resident (through layer 1) for the self loop
            g1k = tc.tile_pool(name="g1keep", bufs=1)
            g1kpool = g1k.__enter__()
            g1_sb = g1kpool.tile([P, NT, DIN], BF16, tag="g1")
            with tc.tile_pool(name="xg", bufs=1) as xgpool:
                x_sb = xgpool.tile([P, NT, DIN], F32, tag="x")
                nc.sync.dma_start(x_sb[:, :, :], xshT[:, :])
                nc.vector.tensor_tensor(
                    g1_sb[:, :, :], x_sb[:, :, :],
                    dinv_sb[:, :].unsqueeze(-1).broadcast_to((P, NT, DIN)),
                    ALU.mult)
                for w in range(NT):
                    store_win(g1_in, g1_sb[:, w, :], w)

            for s in range(NSW):
                nc.gpsimd.collective_compute(
                    "AllGather", ALU.bypass,
                    replica_groups=[list(range(C))],
                    ins=[g1_in[s].ap().opt()], outs=[g1_tbl[s].ap().opt()])

            # ---- remaining constants (overlap the AllGathers) ----
            W1_sb = cpool.tile([P, DH], F32R, tag="W1")
            nc.sync.dma_start(W1_sb[:, :], W1[:, :])
            W2_sb = [cpool.tile([P, DOUT], F32R, tag=f"W2_{k}", name=f"W2_{k}")
                     for k in range(DH // P)]
            for k in range(DH // P):
                nc.sync.dma_start(W2_sb[k][:, :], W2[k * P:(k + 1) * P, :])
            b1_sb = cpool.tile([P, DH], F32, tag="b1")
            nc.sync.dma_start(b1_sb[:, :], b1r[:, :])
            b2_sb = cpool.tile([P, DOUT], F32, tag="b2")
            nc.sync.dma_start(b2_sb[:, :], b2r[:, :])

            dstrel_sb = cpool.tile([P, plan.TOTCH], I16, tag="dstrel")
            nc.sync.dma_start(dstrel_sb[:, :], dstrel[:, :])
            idx_sb = []
            for s in range(NSW):
                t = cpool.tile([P, plan.LS[s] // 16], I16, tag=f"idx{s}",
                               name=f"idx{s}")
                nc.sync.dma_start(t[:, :], idx_d[s][:, :])
                idx_sb.append(t)

            iota_sb = cpool.tile([P, CHMAX, P], I16, tag="iota")
            nc.gpsimd.iota(iota_sb[:, :, :], pattern=[[0, CHMAX], [1, P]],
                           base=0, channel_multiplier=0)
            iota_p = cpool.tile([P, P], I16, tag="iota_p")
            nc.gpsimd.iota(iota_p[:, :], pattern=[[0, P]], base=0,
                           channel_multiplier=1)
            ident = cpool.tile([P, P], BF16, tag="ident")
            nc.vector.tensor_tensor(ident[:, :], iota_sb[:, 0, :], iota_p[:, :],
                                    ALU.is_equal)
            identf = cpool.tile([P, P], F32, tag="identf")
            nc.vector.tensor_tensor(identf[:, :], iota_sb[:, 0, :], iota_p[:, :],
                                    ALU.is_equal)

            # g2 rows stay resident for the layer-2 self-loop contribution
            g2_sb = cpool.tile([P, NT, DOUT], BF16, tag="g2keep")

            # ---- a generic gather/consume pass -------------------------------
            def run_pass(layer, tbls, dim, consume_fn, paypools, bufs,
                         agpts=()):
                chunk_tile = [dict() for _ in range(NSW)]  # q -> (tile, slot)
                heads = [0, 0]
                qrot = [0]

                def issue_next(s):
                    c0, nch, _fw = plan.calls[s][heads[s]]
                    heads[s] += 1
                    pay = paypools[s].tile([P, CALLCH, dim], BF16,
                                           tag=f"pay{layer}_{s}",
                                           name=f"pay{layer}_{s}_{c0}")
                    n = nch * P
                    nc.gpsimd.dma_gather(
                        pay[:, :nch, :], tbls[s][:, :],
                        idx_sb[s][:, c0 * 8:(c0 + nch) * 8],
                        n, n, dim, queue_num=qrot[0] % 4)
                    qrot[0] += 1
                    for j in range(nch):
                        chunk_tile[s][c0 + j] = (pay, j)

                def issue_due(w):
                    while True:
                        # front-load stripe-0 calls up to its (deep) ring so
                        # stripe-0 descriptor generation fills the window in
                        # which the stripe-1 AllGather is still in flight.
                        # Cap at bufs-1 so the next emitted call is stripe-1
                        # (deadlock safety: window w consume needs stripe 1).
                        if heads[0] < min(bufs[0] - 1, len(plan.calls[0])) and \
                                heads[0] - heads[1] < bufs[0] - 1:
                            issue_next(0)
                            continue
                        best = None
                        for s in range(NSW):
                            if heads[s] < len(plan.calls[s]):
                                fw = plan.calls[s][heads[s]][2]
                                if fw <= w + LOOKAHEAD and \
                                        (best is None or fw < best[1]):
                                    best = (s, fw)
                        if best is None:
                            return
                        issue_next(best[0])

                for w in range(NT):
                    issue_due(w)
                    chw = int(CH[w, 0] + CH[w, 1])
                    S = spool.tile([P, CHMAX, P], BF16, tag="S")
                    if chw:
                        nc.vector.tensor_tensor(
                            S[:, :chw, :],
                            dstrel_sb[:, woff[w]:woff[w] + chw]
                            .unsqueeze(-1).broadcast_to((P, chw, P)),
                            iota_sb[:, :chw, :], ALU.is_equal)
                    ps = consume_fn.psum(w)
                    # self contribution from local rows (never gathered)
                    nc.tensor.matmul(ps[:, :], ident[:, :],
                                     consume_fn.selfrows(w),
                                     start=True, stop=(chw == 0))
                    j = 0
                    for s in range(NSW):
                        for q in range(offs[s][w], offs[s][w] + int(CH[w, s])):
                            pay, slot = chunk_tile[s][q]
                            nc.tensor.matmul(ps[:, :], S[:, j, :],
                                             pay[:, slot, :],
                                             start=False, stop=(j == chw - 1))
                            j += 1
                    consume_fn.epilogue(w, ps)
                    if w in agpts:
                        agpts[w]()

            # ---- layer 1 consume + layer-2 producer --------------------------
            with tc.tile_pool(name="pay1a", bufs=PAYBUFS[0]) as paypool1a, \
                 tc.tile_pool(name="pay1b", bufs=PAYBUFS[1]) as paypool1b, \
                 tc.tile_pool(name="ps1", bufs=2, space="PSUM") as pspool1, \
                 tc.tile_pool(name="pt1", bufs=2, space="PSUM") as ptpool1:

                class L1:
                    @staticmethod
                    def psum(w):
                        return pspool1.tile([P, DIN], F32, tag="agg1",
                                            name=f"agg1_{w}")

                    @staticmethod
                    def selfrows(w):
                        return g1_sb[:, w, :]

                    @staticmethod
                    def epilogue(w, ps):
                        aggs = epool.tile([P, DIN], F32, tag="aggs")
                        nc.scalar.activation(aggs[:, :], ps[:, :], AF.Copy,
                                             scale=dinv_sb[:, w:w + 1])
                        pt1 = ptpool1.tile([P, P], F32, tag="pt")
                        nc.tensor.transpose(pt1[:, :], aggs[:, :], identf[:, :])
                        aggT = epool.tile([P, DIN], F32R, tag="aggT")
                        nc.vector.tensor_copy(aggT[:, :], pt1[:, :])
                        ps1 = pspool1.tile([P, DH], F32, tag="gemm1",
                                           name=f"gemm1_{w}")
                        nc.tensor.matmul(ps1[:, :], aggT[:, :], W1_sb[:, :],
                                         start=True, stop=True)
                        v = epool.tile([P, DH], F32, tag="v1")
                        nc.vector.scalar_tensor_tensor(v[:, :], ps1[:, :],
                                                       1.0, b1_sb[:, :],
                                                       ALU.mult, ALU.add)
                        h1 = epool.tile([P, DH], F32, tag="h1")
                        nc.scalar.activation(h1[:, :], v[:, :], AF.Relu)
                        hT = []
                        for k in range(DH // P):
                            pt = ptpool1.tile([P, P], F32, tag="pt")
                            nc.tensor.transpose(pt[:, :],
                                                h1[:, k * P:(k + 1) * P],
                                                identf[:, :])
                            hTk = epool.tile([P, P], F32R, tag=f"hT{k}",
                                             name=f"hT{k}_{w}")
                            nc.vector.tensor_copy(hTk[:, :], pt[:, :])
                            hT.append(hTk)
                        ps2 = pspool1.tile([P, DOUT], F32, tag="gemm2",
                                           name=f"gemm2_{w}")
                        for k in range(DH // P):
                            nc.tensor.matmul(ps2[:, :], hT[k][:, :],
                                             W2_sb[k][:, :],
                                             start=(k == 0),
                                             stop=(k == DH // P - 1))
                        nc.scalar.activation(g2_sb[:, w, :], ps2[:, :], AF.Copy,
                                             scale=dinv_sb[:, w:w + 1])
                        store_win(g2_in, g2_sb[:, w, :], w)

                def ag2(s):
                    def emit():
                        nc.gpsimd.collective_compute(
                            "AllGather", ALU.bypass,
                            replica_groups=[list(range(C))],
                            ins=[g2_in[s].ap().opt()],
                            outs=[g2_tbl[s].ap().opt()])
                    return emit

                # stripe A after w24; stripe B after w48
                agw = {24: ag2(0), 48: ag2(1)}
                run_pass(1, g1_tbl, DIN, L1, (paypool1a, paypool1b),
                         PAYBUFS, agw)

            g1k.__exit__(None, None, None)

            # ---- layer 2 consume + normalize ---------------------------------
            with tc.tile_pool(name="pay2a", bufs=PAYBUFS2[0]) as paypool2a, \
                 tc.tile_pool(name="pay2b", bufs=PAYBUFS2[1]) as paypool2b, \
                 tc.tile_pool(name="ps2", bufs=2, space="PSUM") as pspool2:

                class L2:
                    @staticmethod
                    def psum(w):
                        return pspool2.tile([P, DOUT], F32, tag="agg2",
                                            name=f"agg2_{w}")

                    @staticmethod
                    def selfrows(w):
                        return g2_sb[:, w, :]

                    @staticmethod
                    def epilogue(w, ps):
                        v = epool.tile([P, DOUT], F32, tag="v2")
                        nc.vector.scalar_tensor_tensor(v[:, :], ps[:, :],
                                                       dinv_sb[:, w:w + 1],
                                                       b2_sb[:, :],
                                                       ALU.mult, ALU.add)
                        sq = epool.tile([P, DOUT], F32, tag="sq")
                        ss = epool.tile([P, 1], F32, tag="ss")
                        nc.scalar.activation(sq[:, :], v[:, :], AF.Square,
                                             accum_out=ss[:, :])
                        ssm = epool.tile([P, 1], F32, tag="ssm")
                        nc.vector.tensor_scalar_max(ssm[:, :], ss[:, :], 1e-24)
                        sr = epool.tile([P, 1], F32, tag="sr")
                        nc.scalar.activation(sr[:, :], ssm[:, :], AF.Sqrt)
                        inv = epool.tile([P, 1], F32, tag="inv")
                        nc.vector.reciprocal(inv[:, :], sr[:, :])
                        ot = epool.tile([P, DOUT], F32, tag="ot")
                        nc.scalar.activation(ot[:, :], v[:, :], AF.Copy,
                                             scale=inv[:, 0:1])
                        rows = min(P, NLOC - w * P)
                        nc.sync.dma_start(out[w * P:w * P + rows, :],
                                          ot[:rows, :])

                run_pass(2, g2_tbl, DOUT, L2, (paypool2a, paypool2b),
                         PAYBUFS2)

    nc.compile()
    return nc


_CACHE = {}


def kernel(x, edge_index, W1, b1, W2, b2, **_ignored):
    x = np.asarray(x)
    plan, in_maps = preprocess(x, edge_index, W1, b1, W2, b2)
    key = plan.key()
    if key not in _CACHE:
        _CACHE[key] = build(plan)
    nc = _CACHE[key]
    res = run_bass_kernel_spmd(nc, in_maps, core_ids=list(range(C)))
    return np.concatenate([res.results[c]["out"] for c in range(C)], axis=0)


# revision 16
# speedup vs baseline: 1.0468x; 1.0468x over previous
"""2-layer GCN encoder on 8 Trainium2 NeuronCores (Bass/Tile).

Strategy: nodes sharded across 8 cores; edges partitioned by destination
node; per-layer feature tables AllGathered in two loc-stripes; scatter-add
done locally per dst shard via one-hot matmuls on the tensor engine.

Math: GCNConv's symmetric norm factorizes: norm(e) = dinv[src]*dinv[dst].
With g = dinv * (x @ W), the aggregation is out[d] = dinv[d]*(g[d] +
sum_{e:dst=d} g[src(e)]) + b (the g[d] term is the self loop, applied from
local SBUF data via an identity matmul -- never gathered).

Perf structure:
  - gather calls are 1024 rows each (the ucode max), 4 SWDGE queues
  - self loops use local data (6% fewer gather descriptors)
  - the node table is split into two loc-stripes, each AllGathered
    separately; layer-2 AllGathers are chunked (3 pieces) so they overlap
    the tail of the layer-1 consume loop
  - gather indices are loaded once and reused by both layers
  - layer-2 self rows (g2) stay resident in SBUF from the layer-1 epilogue
"""

import sys

for _p in ("/opt/trn_rl_repo", "/opt/trn_rl_repo/concourse"):
    if _p not in sys.path:
        sys.path.insert(0, _p)

import numpy as np

import concourse.bacc as bacc
import concourse.mybir as mybir
import concourse.tile as tile
from concourse.bass_utils import run_bass_kernel_spmd
from concourse.library_config import mlp as _mlp_lib

F32 = mybir.dt.float32
F32R = mybir.dt.float32r
BF16 = mybir.dt.bfloat16
I16 = mybir.dt.int16
AF = mybir.ActivationFunctionType
ALU = mybir.AluOpType

C = 8          # cores
P = 128        # partitions / window size
NSW = 2        # table stripes
CALLCH = 8     # chunks (of 128 rows) per dma_gather call (ucode max 1024)
LOOKAHEAD = 4    # windows of gather issue-ahead
PAYBUFS = (10, 8)   # layer-1 payload ring depth per stripe
PAYBUFS2 = (12, 8)  # layer-2 payload ring depth per stripe


def _ceil(a, b):
    return (a + b - 1) // b


class Plan:
    """Static structure shared by all cores (program shape)."""

    def __init__(self, N, E, DIN, DH, DOUT):
        self.N, self.E = N, E
        self.DIN, self.DH, self.DOUT = DIN, DH, DOUT
        self.NLOC = N // C
        self.NT = _ceil(self.NLOC, P)
        self.NLOC_PAD = self.NT * P
        assert self.NLOC_PAD % NSW == 0
        self.SW = self.NLOC_PAD // NSW          # stripe width (locs)
        self.STBL = C * self.SW                 # rows per stripe table
        assert self.STBL <= 32768
        self.CH = None        # [NT, NSW] common chunk counts
        self.offs = None      # [NSW][NT] chunk offsets within stream
        self.woff = None      # [NT] dstrel column offset of window w
        self.LS = None        # [NSW] stream rows
        self.TOTCH = None
        self.CHMAX = None
        self.calls = None     # [NSW] list of (chunk0, nch, first_w)

    def key(self):
        return (self.N, self.E, self.CH.tobytes())


def preprocess(x, edge_index, W1, b1, W2, b2):
    """Host-side sharding: integer index metadata only (no float compute on
    the feature data beyond dtype casts / layout of the given tensors)."""
    x = np.asarray(x)
    N, DIN = x.shape
    E = edge_index.shape[1]
    DH = W1.shape[1]
    DOUT = W2.shape[1]
    plan = Plan(N, E, DIN, DH, DOUT)
    NLOC, NT, SW = plan.NLOC, plan.NT, plan.SW

    src = np.asarray(edge_index[0], dtype=np.int64)
    dst = np.asarray(edge_index[1], dtype=np.int64)

    # degree with self loop -- integer graph metadata
    deg = (np.bincount(dst, minlength=N) + 1).astype(np.float32)

    co = dst // NLOC                     # owning core (by dst)
    dl = dst - co * NLOC
    w_of = dl // P
    rel = (dl % P).astype(np.int16)
    sc = src // NLOC
    sl = src - sc * NLOC
    stripe = sl // SW                     # 0/1
    rowidx = (sc * SW + sl - stripe * SW).astype(np.int16)

    # group edges by (core, window, stripe)
    gid = (co * NT + w_of) * NSW + stripe
    order = np.argsort(gid, kind="stable")
    gid_s = gid[order]
    row_s = rowidx[order]
    rel_s = rel[order]
    n_groups = C * NT * NSW
    counts = np.bincount(gid_s, minlength=n_groups).reshape(C, NT, NSW)
    starts = np.concatenate([[0], np.cumsum(counts.reshape(-1))[:-1]]).reshape(
        C, NT, NSW)

    # common chunk structure = max over cores
    CH = np.ceil(counts.max(axis=0) / P).astype(np.int64)      # [NT, NSW]
    plan.CH = CH
    plan.offs = [np.concatenate([[0], np.cumsum(CH[:, s])[:-1]]) for s in range(NSW)]
    plan.LS = [int(CH[:, s].sum()) * P for s in range(NSW)]
    chw = CH.sum(axis=1)
    plan.woff = np.concatenate([[0], np.cumsum(chw)[:-1]])
    plan.TOTCH = int(chw.sum())
    plan.CHMAX = int(chw.max())

    # call packing per stripe (common across cores)
    plan.calls = []
    for s in range(NSW):
        nch = int(CH[:, s].sum())
        calls = []
        c0 = 0
        wof = np.repeat(np.arange(NT), CH[:, s])
        while c0 < nch:
            n = min(CALLCH, nch - c0)
            calls.append((c0, n, int(wof[c0])))
            c0 += n
        plan.calls.append(calls)

    def wrap_idx(a):
        # [n] -> [16, n//16] column-major wrap, replicated x8 -> [128, n//16]
        n = a.shape[0]
        w = a.reshape(n // 16, 16).T
        return np.ascontiguousarray(np.tile(w, (8, 1)))

    in_maps = []
    for c in range(C):
        lo = c * NLOC
        # x, pre-transposed to [128, NT*DIN]: col w*DIN+f, row p = node w*P+p
        xpad = np.zeros((plan.NLOC_PAD, DIN), np.float32)
        xpad[:NLOC] = x[lo:lo + NLOC]
        xshT = np.ascontiguousarray(
            xpad.reshape(NT, P, DIN).transpose(1, 0, 2).reshape(P, NT * DIN))

        degf = np.ones((plan.NLOC_PAD,), np.float32)
        degf[:NLOC] = deg[lo:lo + NLOC]
        degf = np.ascontiguousarray(degf.reshape(NT, P).T)   # [P, NT]

        idxs = [np.zeros((plan.LS[s],), np.int16) for s in range(NSW)]
        drel = np.full((plan.TOTCH * P,), -1, np.int16)
        for w in range(NT):
            for s in range(NSW):
                n = counts[c, w, s]
                st = starts[c, w, s]
                p0 = plan.offs[s][w] * P
                idxs[s][p0:p0 + n] = row_s[st:st + n]
                col0 = plan.woff[w] + (CH[w, 0] if s == 1 else 0)
                drel[col0 * P:col0 * P + n] = rel_s[st:st + n]

        in_maps.append({
            "xshT": xshT,
            "W1": np.asarray(W1, np.float32),
            "W2": np.asarray(W2, np.float32),
            "b1r": np.tile(np.asarray(b1, np.float32)[None, :], (P, 1)),
            "b2r": np.tile(np.asarray(b2, np.float32)[None, :], (P, 1)),
            "degf": degf,
            "idx0": wrap_idx(idxs[0]),
            "idx1": wrap_idx(idxs[1]),
            "dstrel": np.ascontiguousarray(drel.reshape(plan.TOTCH, P).T),
        })
    return plan, in_maps


def build(plan: Plan):
    DIN, DH, DOUT = plan.DIN, plan.DH, plan.DOUT
    NT, SW, STBL = plan.NT, plan.SW, plan.STBL
    NLOC = plan.NLOC
    CH, offs, woff = plan.CH, plan.offs, plan.woff
    CHMAX = plan.CHMAX

    nc = bacc.Bacc("TRN2", target_bir_lowering=False, debug=False, num_devices=C,
                   dynamic_dma_scratch_size=32768, num_swdge_queues=4)

    xshT = nc.dram_tensor("xshT", [P, NT * DIN], F32, kind="ExternalInput")
    W1 = nc.dram_tensor("W1", [DIN, DH], F32R, kind="ExternalInput")
    W2 = nc.dram_tensor("W2", [DH, DOUT], F32R, kind="ExternalInput")
    b1r = nc.dram_tensor("b1r", [P, DH], F32, kind="ExternalInput")
    b2r = nc.dram_tensor("b2r", [P, DOUT], F32, kind="ExternalInput")
    degf = nc.dram_tensor("degf", [P, NT], F32, kind="ExternalInput")
    idx_d = [nc.dram_tensor(f"idx{s}", [P, plan.LS[s] // 16], I16,
                            kind="ExternalInput") for s in range(NSW)]
    dstrel = nc.dram_tensor("dstrel", [P, plan.TOTCH], I16, kind="ExternalInput")
    out = nc.dram_tensor("out", [NLOC, DOUT], F32, kind="ExternalOutput")

    g1_in = [nc.dram_tensor(f"g1_in{s}", [SW, DIN], BF16) for s in range(NSW)]
    g1_tbl = [nc.dram_tensor(f"g1_tbl{s}", [STBL, DIN], BF16, addr_space="Shared")
              for s in range(NSW)]
    g2_in = [nc.dram_tensor(f"g2_in{s}", [SW, DOUT], BF16) for s in range(NSW)]
    g2_tbl = [nc.dram_tensor(f"g2_tbl{s}", [STBL, DOUT], BF16, addr_space="Shared")
              for s in range(NSW)]

    with tile.TileContext(nc) as tc:
        with tc.tile_pool(name="const", bufs=1) as cpool, \
             tc.tile_pool(name="sbuild", bufs=3) as spool, \
             tc.tile_pool(name="epi", bufs=3) as epool:

            nc.gpsimd.load_library(_mlp_lib)

            # ---- degree -> dinv, then x -> g1 table production first (so the
            # stripe AllGathers start as early as possible) ----
            deg_sb = cpool.tile([P, NT], F32, tag="deg")
            nc.sync.dma_start(deg_sb[:, :], degf[:, :])
            sq_sb = cpool.tile([P, NT], F32, tag="sqdeg")
            nc.scalar.activation(sq_sb[:, :], deg_sb[:, :], AF.Sqrt)
            dinv_sb = cpool.tile([P, NT], F32, tag="dinv")
            nc.vector.reciprocal(dinv_sb[:, :], sq_sb[:, :])

            def store_win(g_dst, g_tile, w):
                """Store window w ([P, dim] sbuf rows) into per-stripe dram."""
                r0 = w * P
                s0 = r0 // SW
                s1 = (r0 + P - 1) // SW
                if s0 == s1:
                    nc.sync.dma_start(g_dst[s0][r0 - s0 * SW:r0 - s0 * SW + P, :],
                                      g_tile)
                else:
                    cut = s1 * SW - r0
                    nc.sync.dma_start(g_dst[s0][r0 - s0 * SW:, :], g_tile[:cut, :])
                    nc.sync.dma_start(g_dst[s1][0:P - cut, :], g_tile[cut:, :])

            # g1 rows stay resident (through layer 1) for the self loop
            g1k = tc.tile_pool(name="g1keep", bufs=1)
            g1kpool = g1k.__enter__()
            g1_sb = g1kpool.tile([P, NT, DIN], BF16, tag="g1")
            with tc.tile_pool(name="xg", bufs=1) as xgpool:
                x_sb = xgpool.tile([P, NT, DIN], F32, tag="x")
                nc.sync.dma_start(x_sb[:, :, :], xshT[:, :])
                nc.vector.tensor_tensor(
                    g1_sb[:, :, :], x_sb[:, :, :],
                    dinv_sb[:, :].unsqueeze(-1).broadcast_to((P, NT, DIN)),
                    ALU.mult)
                for w in range(NT):
                    store_win(g1_in, g1_sb[:, w, :], w)

            for s in range(NSW):
                nc.gpsimd.collective_compute(
                    "AllGather", ALU.bypass,
                    replica_groups=[list(range(C))],
                    ins=[g1_in[s].ap().opt()], outs=[g1_tbl[s].ap().opt()])

            # ---- remaining constants (overlap the AllGathers) ----
            W1_sb = cpool.tile([P, DH], F32R, tag="W1")
            nc.sync.dma_start(W1_sb[:, :], W1[:, :])
            W2_sb = [cpool.tile([P, DOUT], F32R, tag=f"W2_{k}", name=f"W2_{k}")
                     for k in range(DH // P)]
            for k in range(DH // P):
                nc.sync.dma_start(W2_sb[k][:, :], W2[k * P:(k + 1) * P, :])
            b1_sb = cpool.tile([P, DH], F32, tag="b1")
            nc.sync.dma_start(b1_sb[:, :], b1r[:, :])
            b2_sb = cpool.tile([P, DOUT], F32, tag="b2")
            nc.sync.dma_start(b2_sb[:, :], b2r[:, :])

            dstrel_sb = cpool.tile([P, plan.TOTCH], I16, tag="dstrel")
            nc.sync.dma_start(dstrel_sb[:, :], dstrel[:, :])
            idx_sb = []
            for s in range(NSW):
                t = cpool.tile([P, plan.LS[s] // 16], I16, tag=f"idx{s}",
                               name=f"idx{s}")
                nc.sync.dma_start(t[:, :], idx_d[s][:, :])
                idx_sb.append(t)

            iota_sb = cpool.tile([P, CHMAX, P], I16, tag="iota")
            nc.gpsimd.iota(iota_sb[:, :, :], pattern=[[0, CHMAX], [1, P]],
                           base=0, channel_multiplier=0)
            iota_p = cpool.tile([P, P], I16, tag="iota_p")
            nc.gpsimd.iota(iota_p[:, :], pattern=[[0, P]], base=0,
                           channel_multiplier=1)
            ident = cpool.tile([P, P], BF16, tag="ident")
            nc.vector.tensor_tensor(ident[:, :], iota_sb[:, 0, :], iota_p[:, :],
                                    ALU.is_equal)
            identf = cpool.tile([P, P], F32, tag="identf")
            nc.vector.tensor_tensor(identf[:, :], iota_sb[:, 0, :], iota_p[:, :],
                                    ALU.is_equal)

            # g2 rows stay resident for the layer-2 self-loop contribution
            g2_sb = cpool.tile([P, NT, DOUT], BF16, tag="g2keep")

            # ---- a generic gather/consume pass -------------------------------
            def run_pass(layer, tbls, dim, consume_fn, paypools, bufs,
                         agpts=()):
                chunk_tile = [dict() for _ in range(NSW)]  # q -> (tile, slot)
                heads = [0, 0]
                qrot = [0]

                def issue_next(s):
                    c0, nch, _fw = plan.calls[s][heads[s]]
                    heads[s] += 1
                    pay = paypools[s].tile([P, CALLCH, dim], BF16,
                                           tag=f"pay{layer}_{s}",
                                           name=f"pay{layer}_{s}_{c0}")
                    n = nch * P
                    nc.gpsimd.dma_gather(
                        pay[:, :nch, :], tbls[s][:, :],
                        idx_sb[s][:, c0 * 8:(c0 + nch) * 8],
                        n, n, dim, queue_num=qrot[0] % 4)
                    qrot[0] += 1
                    for j in range(nch):
                        chunk_tile[s][c0 + j] = (pay, j)

                def issue_due(w):
                    while True:
                        # At pass start, front-load stripe-0 calls up to its
                        # ring depth: stripe-0 descriptor generation then fills
                        # the window in which the stripe-1 AllGather is still
                        # in flight. Cap at bufs-1 so the next emitted call is
                        # stripe-1 (deadlock safety: consume needs stripe 1).
                        if heads[1] == 0 and \
                                heads[0] < min(bufs[0] - 1, len(plan.calls[0])):
                            issue_next(0)
                            continue
                        best = None
                        for s in range(NSW):
                            if heads[s] < len(plan.calls[s]):
                                fw = plan.calls[s][heads[s]][2]
                                if fw <= w + LOOKAHEAD and \
                                        (best is None or fw < best[1]):
                                    best = (s, fw)
                        if best is None:
                            return
                        issue_next(best[0])

                for w in range(NT):
                    issue_due(w)
                    chw = int(CH[w, 0] + CH[w, 1])
                    S = spool.tile([P, CHMAX, P], BF16, tag="S")
                    if chw:
                        nc.vector.tensor_tensor(
                            S[:, :chw, :],
                            dstrel_sb[:, woff[w]:woff[w] + chw]
                            .unsqueeze(-1).broadcast_to((P, chw, P)),
                            iota_sb[:, :chw, :], ALU.is_equal)
                    ps = consume_fn.psum(w)
                    # self contribution from local rows (never gathered)
                    nc.tensor.matmul(ps[:, :], ident[:, :],
                                     consume_fn.selfrows(w),
                                     start=True, stop=(chw == 0))
                    j = 0
                    for s in range(NSW):
                        for q in range(offs[s][w], offs[s][w] + int(CH[w, s])):
                            pay, slot = chunk_tile[s][q]
                            nc.tensor.matmul(ps[:, :], S[:, j, :],
                                             pay[:, slot, :],
                                             start=False, stop=(j == chw - 1))
                            j += 1
                    consume_fn.epilogue(w, ps)
                    if w in agpts:
                        agpts[w]()

            # ---- layer 1 consume + layer-2 producer --------------------------
            with tc.tile_pool(name="pay1a", bufs=PAYBUFS[0]) as paypool1a, \
                 tc.tile_pool(name="pay1b", bufs=PAYBUFS[1]) as paypool1b, \
                 tc.tile_pool(name="ps1", bufs=2, space="PSUM") as pspool1, \
                 tc.tile_pool(name="pt1", bufs=2, space="PSUM") as ptpool1:

                class L1:
                    @staticmethod
                    def psum(w):
                        return pspool1.tile([P, DIN], F32, tag="agg1",
                                            name=f"agg1_{w}")

                    @staticmethod
                    def selfrows(w):
                        return g1_sb[:, w, :]

                    @staticmethod
                    def epilogue(w, ps):
                        aggs = epool.tile([P, DIN], F32, tag="aggs")
                        nc.scalar.activation(aggs[:, :], ps[:, :], AF.Copy,
                                             scale=dinv_sb[:, w:w + 1])
                        pt1 = ptpool1.tile([P, P], F32, tag="pt")
                        nc.tensor.transpose(pt1[:, :], aggs[:, :], identf[:, :])
                        aggT = epool.tile([P, DIN], F32R, tag="aggT")
                        nc.vector.tensor_copy(aggT[:, :], pt1[:, :])
                        ps1 = pspool1.tile([P, DH], F32, tag="gemm1",
                                           name=f"gemm1_{w}")
                        nc.tensor.matmul(ps1[:, :], aggT[:, :], W1_sb[:, :],
                                         start=True, stop=True)
                        v = epool.tile([P, DH], F32, tag="v1")
                        nc.vector.scalar_tensor_tensor(v[:, :], ps1[:, :],
                                                       1.0, b1_sb[:, :],
                                                       ALU.mult, ALU.add)
                        h1 = epool.tile([P, DH], F32, tag="h1")
                        nc.scalar.activation(h1[:, :], v[:, :], AF.Relu)
                        hT = []
                        for k in range(DH // P):
                            pt = ptpool1.tile([P, P], F32, tag="pt")
                            nc.tensor.transpose(pt[:, :],
                                                h1[:, k * P:(k + 1) * P],
                                                identf[:, :])
                            hTk = epool.tile([P, P], F32R, tag=f"hT{k}",
                                             name=f"hT{k}_{w}")
                            nc.vector.tensor_copy(hTk[:, :], pt[:, :])
                            hT.append(hTk)
                        ps2 = pspool1.tile([P, DOUT], F32, tag="gemm2",
                                           name=f"gemm2_{w}")
                        for k in range(DH // P):
                            nc.tensor.matmul(ps2[:, :], hT[k][:, :],
                                             W2_sb[k][:, :],
                                             start=(k == 0),
                                             stop=(k == DH // P - 1))
                        nc.scalar.activation(g2_sb[:, w, :], ps2[:, :], AF.Copy,
                                             scale=dinv_sb[:, w:w + 1])
                        store_win(g2_in, g2_sb[:, w, :], w)

                def ag2(s):
                    def emit():
                        nc.gpsimd.collective_compute(
                            "AllGather", ALU.bypass,
                            replica_groups=[list(range(C))],
                            ins=[g2_in[s].ap().opt()],
                            outs=[g2_tbl[s].ap().opt()])
                    return emit

                # stripe A after w24; stripe B after w48
                agw = {24: ag2(0), 48: ag2(1)}
                run_pass(1, g1_tbl, DIN, L1, (paypool1a, paypool1b),
                         PAYBUFS, agw)

            g1k.__exit__(None, None, None)

            # ---- layer 2 consume + normalize ---------------------------------
            with tc.tile_pool(name="pay2a", bufs=PAYBUFS2[0]) as paypool2a, \
                 tc.tile_pool(name="pay2b", bufs=PAYBUFS2[1]) as paypool2b, \
                 tc.tile_pool(name="ps2", bufs=2, space="PSUM") as pspool2:

                class L2:
                    @staticmethod
                    def psum(w):
                        return pspool2.tile([P, DOUT], F32, tag="agg2",
                                            name=f"agg2_{w}")

                    @staticmethod
                    def selfrows(w):
                        return g2_sb[:, w, :]

                    @staticmethod
                    def epilogue(w, ps):
                        v = epool.tile([P, DOUT], F32, tag="v2")
                        nc.vector.scalar_tensor_tensor(v[:, :], ps[:, :],
                                                       dinv_sb[:, w:w + 1],
                                                       b2_sb[:, :],
                                                       ALU.mult, ALU.add)
                        sq = epool.tile([P, DOUT], F32, tag="sq")
                        ss = epool.tile([P, 1], F32, tag="ss")
                        nc.scalar.activation(sq[:, :], v[:, :], AF.Square,
                                             accum_out=ss[:, :])
                        ssm = epool.tile([P, 1], F32, tag="ssm")
                        nc.vector.tensor_scalar_max(ssm[:, :], ss[:, :], 1e-24)
                        sr = epool.tile([P, 1], F32, tag="sr")
                        nc.scalar.activation(sr[:, :], ssm[:, :], AF.Sqrt)
                        inv = epool.tile([P, 1], F32, tag="inv")
                        nc.vector.reciprocal(inv[:, :], sr[:, :])
                        ot = epool.tile([P, DOUT], F32, tag="ot")
                        nc.scalar.activation(ot[:, :], v[:, :], AF.Copy,
                                             scale=inv[:, 0:1])
                        rows = min(P, NLOC - w * P)
                        nc.sync.dma_start(out[w * P:w * P + rows, :],
                                          ot[:rows, :])

                run_pass(2, g2_tbl, DOUT, L2, (paypool2a, paypool2b),
                         PAYBUFS2)

    nc.compile()
    return nc


_CACHE = {}


def kernel(x, edge_index, W1, b1, W2, b2, **_ignored):
    x = np.asarray(x)
    plan, in_maps = preprocess(x, edge_index, W1, b1, W2, b2)
    key = plan.key()
    if key not in _CACHE:
        _CACHE[key] = build(plan)
    nc = _CACHE[key]
    res = run_bass_kernel_spmd(nc, in_maps, core_ids=list(range(C)))
    return np.concatenate([res.results[c]["out"] for c in range(C)], axis=0)
